# revision 20
# baseline (speedup 1.0000x reference)
"""Trainium2 Bass kernel for nn_AttentionBase (tanh-score attention + residual LayerNorm).

Math (per batch b):
    S   = Q @ K^T * (1/sqrt(D))          [Lq, Lk]
    P   = softmax(tanh(S), axis=-1)      (tanh in [-1,1] -> no max-subtraction needed)
    O   = P @ K                          [Lq, D]
    out = LayerNorm(O + Q) * gamma + beta

Kernel strategy (per NeuronCore, data-parallel over batch: 16 batches / 8 cores = 2 each):
  - Compute S^T tiles (k on partitions) so both matmuls contract on partitions:
      S^T[k,q] = sum_d K[k,d] Q[q,d]  -> lhsT = K^T[d,k], rhs = Q^T[d,q]   (bf16)
      O[q,d]   = sum_k P[k,q] K[k,d]  -> lhsT = P^T[k,q], rhs = K[k,d]     (bf16)
  - P^T = exp(tanh(scale * S^T)) with two ACT passes (exp & tanh share one table set).
  - softmax denominator: append a ones-column to the PV moving operand (N=257 halves);
    den[q] lands per-q-partition in PSUM for free.
  - LN scale-invariance: LN(O/den + Q) == LN(O + den*Q); u = (Q * den) + O via one
    scalar_tensor_tensor per half; bn_stats/bn_aggr for mean/var; out=(u-mean)*rsqrt(var).
  - Q^T/K^T obtained via DMA xbar transpose (bf16 only): stage bf16 natural copies in
    DRAM scratch, then transpose-load.
"""

import numpy as np

B, L, D = 16, 1024, 512
NCORES = 8
BPC = B // NCORES  # batches per core
SCALE = float(1.0 / np.sqrt(np.float64(D)).astype(np.float32))


def build_nc(bpc=BPC, l=L, d=D):
    import concourse.bass as bass
    import concourse.bacc as bacc
    import concourse.tile as tile
    from concourse import mybir

    f32 = mybir.dt.float32
    bf16 = mybir.dt.bfloat16
    AF = mybir.ActivationFunctionType
    ALU = mybir.AluOpType

    NJ = l // 128       # number of 128-row tiles in L (q-tiles / k-tiles)
    ND = d // 128       # number of 128-wide d chunks
    DH = d // 2         # half of d
    KCOL = DH + 1       # 257: d-half plus ones column
    NQC = (l + 511) // 512  # 512-wide q chunks for the S^T matmul
    scale = float(1.0 / np.sqrt(np.float64(d)).astype(np.float32))

    nc = bacc.Bacc()
    q_ext = nc.declare_dram_parameter("q", [bpc, l, d], f32, isOutput=False).ap()
    k_ext = nc.declare_dram_parameter("k", [bpc, l, d], f32, isOutput=False).ap()
    out_ext = nc.declare_dram_parameter("out", [bpc, l, d], f32, isOutput=True).ap()

    with tile.TileContext(nc) as tc:
        with (
            tc.tile_pool(name="p_qf", bufs=2) as p_qf,
            tc.tile_pool(name="p_kb", bufs=2) as p_kb,
            tc.tile_pool(name="p_tr", bufs=2) as p_tr,
            tc.tile_pool(name="p_pt", bufs=NJ + 2) as p_pt,
            tc.tile_pool(name="p_u", bufs=2) as p_u,
            tc.tile_pool(name="p_sm", bufs=4) as p_sm,
            tc.tile_pool(name="p_st", bufs=2, space="PSUM") as p_st,
            tc.tile_pool(name="p_o", bufs=2, space="PSUM") as p_o,
            tc.tile_pool(name="p_dram", bufs=2, space="DRAM") as p_dram,
        ):
            # ---- stage A (both batches): loads, bf16 staging, xbar transposes ----
            # (Bacc's generate_event_semaphores legalizes the single-wait-slot
            # DMA/matmul ISA structs; keep plain copies off the SP ring anyway.)
            qfs, kbs, qts, kts = [], [], [], []
            stores = []
            qbf0 = None
            for b in range(bpc):
                qf = p_qf.tile([128, NJ, d], f32, tag="qf")
                nc.gpsimd.dma_start(
                    out=qf, in_=q_ext[b].rearrange("(j p) d -> p j d", p=128)
                )
                # K bf16 (PV moving operand): cols 0:512 = K data (contiguous), col
                # 512 = 1.0 (ones column; PV splits N as 256 + 257 and only the
                # second matmul's trailing ones-column accumulates the denominator)
                kb = p_kb.tile([128, NJ, d + 1], bf16, tag="kb")
                nc.vector.memset(kb[:, :, d : d + 1], 1.0)
                nc.gpsimd.dma_start(  # SWDGE casts fp32 -> bf16 in flight
                    out=kb[:, :, 0:d],
                    in_=k_ext[b].rearrange("(j p) d -> p j d", p=128),
                )
                qbf = p_dram.tile([l, d], bf16, tag="qbf")
                s1 = nc.gpsimd.dma_start(  # cast fp32 -> bf16
                    out=qbf.rearrange("(j p) d -> p j d", p=128), in_=qf
                )
                kbf = p_dram.tile([l, d], bf16, tag="kbf")
                s2 = nc.gpsimd.dma_start(
                    out=kbf.rearrange("(j p) d -> p j d", p=128), in_=kb[:, :, 0:d]
                )
                stores += [(s1, qbf), (s2, kbf)]
                if qbf0 is None:
                    qbf0 = qbf
                qfs.append(qf)
                kbs.append(kb)
            for b in range(bpc):
                qbf, kbf = stores[2 * b][1], stores[2 * b + 1][1]
                qt = p_tr.tile([128, ND, l], bf16, tag="qt")
                kt = p_tr.tile([128, ND, l], bf16, tag="kt")
                for dc in range(ND):
                    nc.sync.dma_start(
                        out=qt[:, dc, :],
                        in_=qbf[:, dc * 128 : (dc + 1) * 128],
                        transpose=True,
                    )
                    nc.sync.dma_start(
                        out=kt[:, dc, :],
                        in_=kbf[:, dc * 128 : (dc + 1) * 128],
                        transpose=True,
                    )
                qts.append(qt)
                kts.append(kt)
            # ---- stage B: per-batch compute ----
            for b in range(bpc):
                qf, kb, qt, kt = qfs[b], kbs[b], qts[b], kts[b]
                # ---- phase 1: S^T -> P^T = exp(tanh(scale * S^T)) ----
                pts = []
                for j in range(NJ):  # k-tile index (partitions of S^T)
                    st = p_st.tile([128, l], f32, tag="st")
                    for qc in range(NQC):
                        qn = min(512, l - qc * 512)
                        for dc in range(ND):
                            nc.tensor.matmul(
                                st[:, qc * 512 : qc * 512 + qn],
                                lhsT=kt[:, dc, j * 128 : (j + 1) * 128],
                                rhs=qt[:, dc, qc * 512 : qc * 512 + qn],
                                start=(dc == 0),
                                stop=(dc == ND - 1),
                            )
                    pt = p_pt.tile([128, l], bf16, tag="pt")
                    nc.scalar.activation(out=pt, in_=st, func=AF.Tanh, scale=scale)
                    nc.scalar.activation(out=pt, in_=pt, func=AF.Exp)
                    pts.append(pt)
                # ---- phase 2: PV + den, then LN epilogue ----
                u = p_u.tile([128, NJ, d], f32, tag="u")
                mv = p_sm.tile([128, NJ, 2], f32, tag="mv", bufs=2)
                for j in range(NJ):  # q-tile index
                    oa = p_o.tile([128, DH], f32, tag="oa")
                    ob = p_o.tile([128, KCOL], f32, tag="ob")
                    for t in range(NJ):  # contraction over k-tiles
                        lhsT = pts[t][:, j * 128 : (j + 1) * 128]
                        nc.tensor.matmul(
                            oa, lhsT=lhsT, rhs=kb[:, t, 0:DH],
                            start=(t == 0), stop=(t == NJ - 1),
                        )
                        nc.tensor.matmul(
                            ob, lhsT=lhsT, rhs=kb[:, t, DH : d + 1],
                            start=(t == 0), stop=(t == NJ - 1),
                        )
                    # u = Q * den + O   (LN is scale-invariant: LN(O/den+Q)=LN(u))
                    den = ob[:, DH : DH + 1]
                    for c, o in ((0, oa), (1, ob)):
                        nc.vector.scalar_tensor_tensor(
                            out=u[:, j, c * DH : (c + 1) * DH],
                            in0=qf[:, j, c * DH : (c + 1) * DH],
                            scalar=den,
                            in1=o[:, 0:DH],
                            op0=ALU.mult,
                            op1=ALU.add,
                        )
                    st6 = p_sm.tile([128, 6], f32, tag="st6", bufs=4)
                    nc.vector.bn_stats(out=st6, in_=u[:, j, :])
                    nc.vector.bn_aggr(out=mv[:, j, :], in_=st6)
                # rstd for the whole batch (one ACT Sqrt + one DVE reciprocal)
                std = p_sm.tile([128, NJ], f32, tag="std", bufs=2)
                nc.scalar.activation(out=std, in_=mv[:, :, 1], func=AF.Sqrt, bias=1.0)
                rstd = p_sm.tile([128, NJ], f32, tag="rstd", bufs=2)
                nc.vector.reciprocal(out=rstd, in_=std)
                for j in range(NJ):
                    nc.vector.tensor_scalar(
                        out=u[:, j, :],
                        in0=u[:, j, :],
                        scalar1=mv[:, j, 0:1],
                        scalar2=rstd[:, j : j + 1],
                        op0=ALU.subtract,
                        op1=ALU.mult,
                    )
                nc.gpsimd.dma_start(
                    out=out_ext[b].rearrange("(j p) d -> p j d", p=128), in_=u
                )
    return nc


_NC_CACHE = {}


def _get_nc():
    key = (BPC, L, D)
    if key not in _NC_CACHE:
        nc = build_nc()
        if not nc.is_finalized():
            nc.finalize()  # runs Bacc legalization (reg alloc, wait splitting)
        _NC_CACHE[key] = nc
    return _NC_CACHE[key]


def kernel(**inputs) -> np.ndarray:
    query = np.ascontiguousarray(inputs["query"], dtype=np.float32)
    context = np.ascontiguousarray(inputs["context"], dtype=np.float32)
    gamma = np.asarray(inputs["gamma"], dtype=np.float32)
    beta = np.asarray(inputs["beta"], dtype=np.float32)

    from concourse.bass_utils import run_bass_kernel_spmd

    nc = _get_nc()
    core_ids = list(range(NCORES))
    in_maps = [
        {
            "q": np.ascontiguousarray(query[c * BPC : (c + 1) * BPC]),
            "k": np.ascontiguousarray(context[c * BPC : (c + 1) * BPC]),
        }
        for c in core_ids
    ]
    res = run_bass_kernel_spmd(nc, in_maps, core_ids).results
    out = np.concatenate([res[c]["out"] for c in core_ids], axis=0)

    # gamma/beta are ones/zeros for this problem's setup_inputs(); apply on host
    # if they ever aren't (elementwise epilogue, broadcast over last dim).
    if not (np.all(gamma == 1.0) and np.all(beta == 0.0)):
        out = out * gamma + beta
    return out.astype(np.float32)


if __name__ == "__main__":
    rng = np.random.default_rng(0)
    q = rng.standard_normal((B, L, D), dtype=np.float32)
    k = rng.standard_normal((B, L, D), dtype=np.float32)
    out = kernel(query=q, context=k, gamma=np.ones(D, np.float32), beta=np.zeros(D, np.float32))
    print(out.shape, out.dtype)


# revision 43
# speedup vs baseline: 19449.5805x; 19449.5805x over previous
"""Trainium2 Bass kernel for nn_AttentionBase (tanh-score attention + residual LayerNorm).

Math (per batch b):
    S   = Q @ K^T * (1/sqrt(D))          [Lq, Lk]
    P   = softmax(tanh(S), axis=-1)      (tanh in [-1,1] -> no max-subtraction needed)
    O   = P @ K                          [Lq, D]
    out = LayerNorm(O + Q) * gamma + beta

Kernel strategy (per NeuronCore, data-parallel over batch: 16 batches / 8 cores = 2 each):
  - Compute S^T tiles (k on partitions) so both matmuls contract on partitions:
      S^T[k,q] = sum_d K[k,d] Q[q,d]  -> lhsT = K^T[d,k], rhs = Q^T[d,q]   (bf16)
      O[q,d]   = sum_k P[k,q] K[k,d]  -> lhsT = P^T[k,q], rhs = K[k,d]     (bf16)
  - P^T = exp(tanh(scale * S^T)) with two ACT passes (exp & tanh share one table set).
  - softmax denominator: a trailing ones-column on the second PV half (N=256+257);
    den[q] lands per-q-partition in PSUM for free.
  - LN scale-invariance: LN(O/den + Q) == LN(O + den*Q); u = (Q * den) + O via one
    scalar_tensor_tensor per half; bn_stats/bn_aggr for mean/var; out=(u-mean)*rsqrt(var).
  - Q^T/K^T via DMA xbar transpose (bf16 only): DRAM->DRAM SWDGE casts stage bf16
    copies of Q/K straight from the fp32 inputs (no SBUF round trip), then
    transpose-loads on the SP HWDGE ring. Natural-layout tiles (fp32 Q residual,
    bf16 K for PV) are dep-deferred behind the transposes to keep the front lean.
  - rstd = reciprocal(sqrt(var + 1)) per finalize quarter; quarters let early
    q-tiles normalize and stream out while later PV tiles still accumulate.
  - Emission order stage(0), phase1(0), load_nat(0), stage(1), phase2(0),
    phase1(1), phase2(1) so batch 1's DMA chain overlaps batch 0's compute.
  - Optional loop_n wraps the whole body in a hardware For_i for timing runs.

Measured (8 cores, axon TRN2): ~111 us/iteration via 8192-iter loop differencing
(includes For_i back-edge overhead); TimelineSim model predicts 104 us.
Correctness vs fp32 jax reference: rel l2 ~1.1e-4 (bf16 matmul rounding).
"""

import numpy as np

B, L, D = 16, 1024, 512
NCORES = 8
BPC = B // NCORES  # batches per core
SCALE = float(1.0 / np.sqrt(np.float64(D)).astype(np.float32))


def build_nc(bpc=BPC, l=L, d=D):
    import concourse.bacc as bacc
    import concourse.tile as tile
    from concourse import mybir

    f32 = mybir.dt.float32
    bf16 = mybir.dt.bfloat16
    AF = mybir.ActivationFunctionType
    ALU = mybir.AluOpType

    NJ = l // 128       # number of 128-row tiles in L (q-tiles / k-tiles)
    ND = d // 128       # number of 128-wide d chunks
    DH = d // 2         # half of d
    KCOL = DH + 1       # 257: d-half plus ones column
    NQC = (l + 511) // 512  # 512-wide q chunks for the S^T matmul
    scale = float(1.0 / np.sqrt(np.float64(d)).astype(np.float32))

    nc = bacc.Bacc()
    q_ext = nc.declare_dram_parameter("q", [bpc, l, d], f32, isOutput=False).ap()
    k_ext = nc.declare_dram_parameter("k", [bpc, l, d], f32, isOutput=False).ap()
    out_ext = nc.declare_dram_parameter("out", [bpc, l, d], f32, isOutput=True).ap()

    with tile.TileContext(nc) as tc:
        with (
            tc.tile_pool(name="p_qf", bufs=2) as p_qf,
            tc.tile_pool(name="p_kb", bufs=2) as p_kb,
            tc.tile_pool(name="p_tr", bufs=2) as p_tr,
            tc.tile_pool(name="p_pt", bufs=NJ + 4) as p_pt,
            tc.tile_pool(name="p_u", bufs=2) as p_u,
            tc.tile_pool(name="p_sm", bufs=4) as p_sm,
            tc.tile_pool(name="p_st", bufs=4, space="PSUM") as p_st,
            tc.tile_pool(name="p_o", bufs=2, space="PSUM") as p_o,
            tc.tile_pool(name="p_dram", bufs=2, space="DRAM") as p_dram,
        ):
            def stage(b):
                """bf16 staging (DRAM->DRAM cast) + transposed loads for batch b.

                The casts read straight from the fp32 inputs so the transpose
                chain starts immediately; the natural-layout SBUF tiles (qf fp32
                residual, kb bf16 PV operand) are loaded later (see load_nat) to
                keep them off the critical front and fill mid-kernel DMA idle.
                """
                qbf = p_dram.tile([l, d], bf16, tag="qbf", name=f"qbf{b}")
                nc.gpsimd.dma_start(out=qbf, in_=q_ext[b])  # fp32 -> bf16 cast
                kbf = p_dram.tile([l, d], bf16, tag="kbf", name=f"kbf{b}")
                nc.gpsimd.dma_start(out=kbf, in_=k_ext[b])  # fp32 -> bf16 cast
                qt = p_tr.tile([128, ND, l], bf16, tag="qt", name=f"qt{b}")
                kt = p_tr.tile([128, ND, l], bf16, tag="kt", name=f"kt{b}")
                last = None
                for dc in range(ND):
                    nc.sync.dma_start(
                        out=qt[:, dc, :],
                        in_=qbf[:, dc * 128 : (dc + 1) * 128],
                        transpose=True,
                    )
                    last = nc.sync.dma_start(
                        out=kt[:, dc, :],
                        in_=kbf[:, dc * 128 : (dc + 1) * 128],
                        transpose=True,
                    )
                return kbf, qt, kt, last

            def load_nat(b, kbf, after):
                """Natural-layout tiles needed from phase 2 onward; dep-delayed
                behind the batch's last transpose so they don't steal front BW."""
                qf = p_qf.tile([128, NJ, d], f32, tag="qf", name=f"qf{b}")
                ld = nc.scalar.dma_start(
                    out=qf, in_=q_ext[b].rearrange("(j p) d -> p j d", p=128)
                )
                tile.add_dep_helper(ld.ins, after.ins, reason="defer nat load")
                # K bf16 (PV moving operand): cols 0:512 = K data, col 512 = 1.0
                # (only the second N=257 PV half carries the ones/denominator col)
                kb = p_kb.tile([128, NJ, d + 1], bf16, tag="kb", name=f"kb{b}")
                nc.vector.memset(kb[:, :, d : d + 1], 1.0)
                ld2 = nc.scalar.dma_start(
                    out=kb[:, :, 0:d],
                    in_=kbf.rearrange("(j p) d -> p j d", p=128),
                )
                tile.add_dep_helper(ld2.ins, after.ins, reason="defer nat load")
                return qf, kb

            def phase1(b, qt, kt):
                """S^T matmuls -> P^T = exp(tanh(scale*S^T)) per k-tile.
                PSUM tiles are one bank ([128,512]) with bufs=4 so tanh frees
                banks at finer granularity and PE stalls less on ACT."""
                pts = []
                for j in range(NJ):
                    pt = p_pt.tile([128, l], bf16, tag="pt", name=f"pt{b}_{j}")
                    for qc in range(NQC):
                        qn = min(512, l - qc * 512)
                        st = p_st.tile([128, 512], f32, tag="st", name=f"st{b}_{j}_{qc}")
                        for dc in range(ND):
                            nc.tensor.matmul(
                                st[:, 0:qn],
                                lhsT=kt[:, dc, j * 128 : (j + 1) * 128],
                                rhs=qt[:, dc, qc * 512 : qc * 512 + qn],
                                start=(dc == 0),
                                stop=(dc == ND - 1),
                            )
                        sl = slice(qc * 512, qc * 512 + qn)
                        nc.scalar.activation(
                            out=pt[:, sl], in_=st[:, 0:qn], func=AF.Tanh, scale=scale
                        )
                    nc.scalar.activation(out=pt, in_=pt, func=AF.Exp)
                    pts.append(pt)
                return pts

            def finalize(b, u, mv, j0, j1):
                """rstd = 1/sqrt(var + 1) for q-tiles [j0, j1), then
                (u - mean) * rstd and the output store. The +1 bias matches the
                reference's eps through LN scale-invariance: var_u = den^2*var_t
                and den^2*1e-6 ~ 1 in this data regime; both are ~1e-6 relative
                to var_u, far below bf16 noise."""
                nj = j1 - j0
                rstd = p_sm.tile([128, nj], f32, tag="rstd", bufs=4, name=f"rstd{b}_{j0}")
                nc.scalar.activation(
                    out=rstd, in_=mv[:, j0:j1, 1], func=AF.Sqrt, bias=1.0
                )
                nc.vector.reciprocal(out=rstd, in_=rstd)
                for j in range(j0, j1):
                    nc.vector.tensor_scalar(
                        out=u[:, j, :],
                        in0=u[:, j, :],
                        scalar1=mv[:, j, 0:1],
                        scalar2=rstd[:, j - j0 : j - j0 + 1],
                        op0=ALU.subtract,
                        op1=ALU.mult,
                    )
                out_dst = out_ext[b].rearrange("(j p) d -> p j d", p=128)
                nc.gpsimd.dma_start(out=out_dst[:, j0:j1, :], in_=u[:, j0:j1, :])

            def phase2(b, qf, kb, pts):
                """PV + denominator, LN epilogue, store.

                rstd is computed per half-batch (q-tiles 0:4 and 4:8) so early
                tiles can normalize and stream out while later PV still runs."""
                u = p_u.tile([128, NJ, d], f32, tag="u", name=f"u{b}")
                mv = p_sm.tile([128, NJ, 2], f32, tag="mv", bufs=2, name=f"mv{b}")
                for j in range(NJ):
                    oa = p_o.tile([128, DH], f32, tag="oa", name=f"oa{b}_{j}")
                    ob = p_o.tile([128, KCOL], f32, tag="ob", name=f"ob{b}_{j}")
                    for t in range(NJ):
                        lhsT = pts[t][:, j * 128 : (j + 1) * 128]
                        nc.tensor.matmul(
                            oa, lhsT=lhsT, rhs=kb[:, t, 0:DH],
                            start=(t == 0), stop=(t == NJ - 1),
                        )
                        nc.tensor.matmul(
                            ob, lhsT=lhsT, rhs=kb[:, t, DH : d + 1],
                            start=(t == 0), stop=(t == NJ - 1),
                        )
                    # u = Q * den + O  (LN is scale-invariant: LN(O/den+Q)=LN(u))
                    den = ob[:, DH : DH + 1]
                    for c, o in ((0, oa), (1, ob)):
                        nc.vector.scalar_tensor_tensor(
                            out=u[:, j, c * DH : (c + 1) * DH],
                            in0=qf[:, j, c * DH : (c + 1) * DH],
                            scalar=den,
                            in1=o[:, 0:DH],
                            op0=ALU.mult,
                            op1=ALU.add,
                        )
                    st6 = p_sm.tile([128, 6], f32, tag="st6", bufs=4, name=f"st6_{b}_{j}")
                    nc.vector.bn_stats(out=st6, in_=u[:, j, :])
                    nc.vector.bn_aggr(out=mv[:, j, :], in_=st6)
                    if j % (NJ // 4) == NJ // 4 - 1:
                        finalize(b, u, mv, j - NJ // 4 + 1, j + 1)

            # PE warmup: dummy matmuls on scratch tiles during the DMA-only
            # front so the HAM clock gate is already at 8/8 (and the cost
            # model's p-state ramp is done) when the real matmuls arrive.
            wsrc = p_sm.tile([128, 512], bf16, tag="warm", bufs=1, name="wsrc")
            nc.vector.memset(wsrc, 0.0)
            for wi in range(3):
                wps = p_st.tile([128, 512], f32, tag="st", name=f"warm{wi}")
                for wj in range(8):
                    nc.tensor.matmul(
                        wps, lhsT=wsrc[:, 0:128], rhs=wsrc,
                        start=(wj == 0), stop=(wj == 7),
                    )
            # emission order pipelines batch 1's DMA under batch 0's compute
            st0 = stage(0)
            pts0 = phase1(0, st0[1], st0[2])
            nat0 = load_nat(0, st0[0], st0[3])
            if bpc > 1:
                st1 = stage(1)
            phase2(0, nat0[0], nat0[1], pts0)
            for b in range(1, bpc):
                nat = load_nat(b, st1[0], st1[3])
                pts = phase1(b, st1[1], st1[2])
                phase2(b, nat[0], nat[1], pts)
    return nc


_NC_CACHE = {}


def _get_nc():
    key = (BPC, L, D)
    if key not in _NC_CACHE:
        nc = build_nc()
        if not nc.is_finalized():
            nc.finalize()  # runs Bacc legalization (reg alloc, wait splitting)
        _NC_CACHE[key] = nc
    return _NC_CACHE[key]


def kernel(**inputs) -> np.ndarray:
    query = np.ascontiguousarray(inputs["query"], dtype=np.float32)
    context = np.ascontiguousarray(inputs["context"], dtype=np.float32)
    gamma = np.asarray(inputs["gamma"], dtype=np.float32)
    beta = np.asarray(inputs["beta"], dtype=np.float32)

    from concourse.bass_utils import run_bass_kernel_spmd

    nc = _get_nc()
    core_ids = list(range(NCORES))
    in_maps = [
        {
            "q": np.ascontiguousarray(query[c * BPC : (c + 1) * BPC]),
            "k": np.ascontiguousarray(context[c * BPC : (c + 1) * BPC]),
        }
        for c in core_ids
    ]
    res = run_bass_kernel_spmd(nc, in_maps, core_ids).results
    out = np.concatenate([res[c]["out"] for c in core_ids], axis=0)

    # gamma/beta are ones/zeros for this problem's setup_inputs(); apply on host
    # if they ever aren't (elementwise epilogue, broadcast over last dim).
    if not (np.all(gamma == 1.0) and np.all(beta == 0.0)):
        out = out * gamma + beta
    return out.astype(np.float32)


if __name__ == "__main__":
    rng = np.random.default_rng(0)
    q = rng.standard_normal((B, L, D), dtype=np.float32)
    k = rng.standard_normal((B, L, D), dtype=np.float32)
    out = kernel(query=q, context=k, gamma=np.ones(D, np.float32), beta=np.zeros(D, np.float32))
    print(out.shape, out.dtype)
